# revision 30
# baseline (speedup 1.0000x reference)
"""Trainium2 Bass kernel for dense sigmoid-masked causal attention.

Problem (full shapes):
    x [B=2, N=2048, D=2048], W_qkv [D, 3D], b_qkv [3D], W_out [D, D],
    b_out [D], causal_mask [H=16, N, N]
    out = softmax((q k^T / sqrt(hd)) * sigmoid(mask)) v @ W_out + b_out

Sharding over 8 NeuronCores: 2-way data parallel on batch x 4-way tensor
parallel on heads (4 heads per core). Each core computes its partial
out-projection (its 4 heads' contribution, including b_out/4); the host sums
the 4 partials per batch element.

Device-side layout ("transposed scores" orientation):
    - host uploads x[b]^T, per-head-group W_qkv columns (q block pre-scaled by
      1/sqrt(hd)), mask^T per head; all in bf16 (fp32 accumulation in PSUM).
    - qT/kT tiles [hd=128, N] come directly out of the qkv^T projection.
    - scores^T tiles [keys, queries] feed attn@v with v in natural layout,
      with no on-device transposes anywhere.
    - softmax denominator = ones-vector matmul over keys (partition dim);
      normalization is applied to out^T via a PE ones-broadcast of 1/denom.
    - sigmoid/exp run as one whole-group ACT op each, batched so the ACT
      engine doesn't thrash its function tables (sigmoid and exp live in
      different ACT LUT tables; a switch costs ~1.3us).
    - biases enter as rank-1 (K=1) matmul updates (skipped if all-zero).
"""

import functools

import numpy as np

B = 2
N = 2048
D = 2048
H = 16
HD = 128
HPC = 4  # heads per core
NCORES = 8
KC = D // 128  # 16 contraction chunks
ALPHA = 1.0 / float(np.sqrt(HD))


@functools.lru_cache(maxsize=4)
def _build_program(zero_bias: bool, repeat: int = 1):
    import concourse.bass as bass  # noqa: F401
    import concourse.mybir as mybir
    import concourse.tile as tile
    from concourse import bacc

    from concourse.tile import add_dep_helper

    f32 = mybir.dt.float32
    bf16 = mybir.dt.bfloat16
    Act = mybir.ActivationFunctionType

    # Bacc (not plain Bass): its compile() pass converts Tile's multi-sem
    # waits into event semaphores — walrus rejects raw multi-wait
    # instructions ("Too many sync wait commands").
    nc = bacc.Bacc("TRN2", target_bir_lowering=False, debug=False)

    xT_d = nc.declare_dram_parameter("xT", [D, N], bf16, isOutput=False)
    wqkv_d = nc.declare_dram_parameter("wqkv", [D, 3 * HPC * HD], bf16, isOutput=False)
    bqkv_d = nc.declare_dram_parameter("bqkv", [1, 3 * HPC * HD], bf16, isOutput=False)
    maskT_d = nc.declare_dram_parameter("maskT", [HPC, N, N], bf16, isOutput=False)
    wout_d = nc.declare_dram_parameter("wout", [HPC * HD, D], bf16, isOutput=False)
    bout_d = nc.declare_dram_parameter("bout", [1, D], bf16, isOutput=False)
    out_d = nc.declare_dram_parameter("out", [N, D], f32, isOutput=True)

    QKW = HPC * HD  # 512: width of the q (and k, and v) column block per core

    with tile.TileContext(nc) as tc:
        with tc.tile_pool(name="const", bufs=1) as const_pool:
            ones_bf = const_pool.tile([128, 512], bf16)
            nc.vector.memset(ones_bf, 1.0)
            ones_f32 = const_pool.tile([1, 128], f32)
            nc.vector.memset(ones_f32, 1.0)

            # ---- persistent SBUF tensors (live across phases) ----
            # `repeat` re-runs the whole pipeline back-to-back inside one
            # NEFF — used only by the timing harness to difference away the
            # fixed per-execution dispatch overhead.
            for _rep in range(repeat):
                _emit_pipeline(
                    nc, tc, mybir, add_dep_helper, zero_bias, ones_bf, ones_f32,
                    xT_d, wqkv_d, bqkv_d, maskT_d, wout_d, bout_d, out_d,
                )

    nc.compile()
    return nc


def _emit_pipeline(
    nc, tc, mybir, add_dep_helper, zero_bias, ones_bf, ones_f32,
    xT_d, wqkv_d, bqkv_d, maskT_d, wout_d, bout_d, out_d,
):
    import concourse.tile as tile  # noqa: F401

    f32 = mybir.dt.float32
    bf16 = mybir.dt.bfloat16
    Act = mybir.ActivationFunctionType
    QKW = HPC * HD

    if True:  # preserved indentation of the original phase body
        if True:
            with tc.tile_pool(name="persist", bufs=1) as persist:
                # qkT[c]: c in 0..3 -> q^T per head (pre-scaled), 4..7 -> k^T
                qk_sb = [
                    persist.tile([128, N], bf16, name=f"qk_sb{c}") for c in range(8)
                ]
                # v in natural layout: [token-part, chunk, 4 heads * 128]
                v_sb = persist.tile([128, KC, QKW], bf16)
                # out^T per head, normalized: [hd, N]
                oT_sb = [
                    persist.tile([128, N], bf16, name=f"oT_sb{h}") for h in range(HPC)
                ]

                # ================= Phase 1: qkv projection =================
                with (
                    tc.tile_pool(name="p1w", bufs=1) as p1w,
                    tc.tile_pool(name="p1ps", bufs=4, space="PSUM") as p1ps,
                ):
                    xT_sb = p1w.tile([128, KC, N], bf16)
                    wqkv_sb = p1w.tile([128, KC, 3 * QKW], bf16)
                    xT_r = xT_d.rearrange("(c p) n -> p c n", p=128)
                    wqkv_r = wqkv_d.rearrange("(c p) n -> p c n", p=128)
                    p1_dmas = []
                    for kc in range(KC):
                        p1_dmas.append(
                            nc.sync.dma_start(out=xT_sb[:, kc, :], in_=xT_r[:, kc, :])
                        )
                        p1_dmas.append(
                            nc.sync.dma_start(
                                out=wqkv_sb[:, kc, :], in_=wqkv_r[:, kc, :]
                            )
                        )
                    if not zero_bias:
                        bqkv_sb = p1w.tile([1, 3 * QKW], bf16)
                        p1_dmas.append(nc.sync.dma_start(out=bqkv_sb, in_=bqkv_d[:, :]))

                    # q^T and k^T: out[cols, tokens]; W is the stationary side.
                    for c in range(8):
                        for t in range(4):  # 512-token chunks
                            ps = p1ps.tile([128, 512], f32, name="p1ps_t")
                            for kc in range(KC):
                                nc.tensor.matmul(
                                    ps,
                                    lhsT=wqkv_sb[:, kc, c * 128 : (c + 1) * 128],
                                    rhs=xT_sb[:, kc, t * 512 : (t + 1) * 512],
                                    start=(kc == 0),
                                    stop=(kc == KC - 1) and zero_bias,
                                )
                            if not zero_bias:
                                # bias: bias_col (M) x ones_row (N)
                                nc.tensor.matmul(
                                    ps,
                                    lhsT=bqkv_sb[0:1, c * 128 : (c + 1) * 128],
                                    rhs=ones_bf[0:1, 0:512],
                                    start=False,
                                    stop=True,
                                )
                            nc.vector.tensor_copy(
                                qk_sb[c][:, t * 512 : (t + 1) * 512], ps
                            )

                    # v in natural layout: x^T is the stationary side.
                    for t in range(16):  # 128-token chunks
                        ps = p1ps.tile([128, 512], f32, name="p1ps_t")
                        for kc in range(KC):
                            nc.tensor.matmul(
                                ps,
                                lhsT=xT_sb[:, kc, t * 128 : (t + 1) * 128],
                                rhs=wqkv_sb[:, kc, 2 * QKW : 3 * QKW],
                                start=(kc == 0),
                                stop=(kc == KC - 1) and zero_bias,
                            )
                        if not zero_bias:
                            nc.tensor.matmul(
                                ps,
                                lhsT=ones_bf[0:1, 0:128],
                                rhs=bqkv_sb[0:1, 2 * QKW : 3 * QKW],
                                start=False,
                                stop=True,
                            )
                        nc.vector.tensor_copy(v_sb[:, t, :], ps)

                # Collapse cross-phase SBUF-reuse waits into one barrier
                # (walrus rejects instructions with too many sem-wait
                # conditions). The phase-1 DMAs land via all 8 HW-DGE queues,
                # so first funnel their completion into the sync engine a few
                # at a time (each nop carries only a handful of sem waits);
                # the barrier then needs waits only on the compute engines.
                for i in range(0, len(p1_dmas), 3):
                    jn = nc.sync.nop()
                    for d in p1_dmas[i : i + 3]:
                        add_dep_helper(jn.ins, d.ins, sync=True)
                tc.strict_bb_all_engine_barrier()

                # ===== Phase 2: attention + fused out-projection per qc =====
                # qc-outer so each 512-query stripe finishes all 4 heads and
                # immediately flows into its out-projection; the out-proj
                # matmuls/copies/stores hide under the next stripe's ACT work.
                with (
                    tc.tile_pool(name="p2mask", bufs=2) as p2mask,
                    tc.tile_pool(name="p2attn", bufs=1) as p2attn,
                    tc.tile_pool(name="p2sig", bufs=2) as p2sig,
                    tc.tile_pool(name="p2mskd", bufs=2) as p2mskd,
                    tc.tile_pool(name="p2r", bufs=2) as p2r,
                    tc.tile_pool(name="p2w", bufs=1) as p2w,
                    tc.tile_pool(name="p3s", bufs=3) as p3s,
                    tc.tile_pool(name="sps", bufs=3, space="PSUM") as spsp,
                    tc.tile_pool(name="dps", bufs=1, space="PSUM") as dpsp,
                    tc.tile_pool(name="bps", bufs=1, space="PSUM") as bpsp,
                    tc.tile_pool(name="ops", bufs=2, space="PSUM") as opsp,
                    tc.tile_pool(name="p3ps", bufs=1, space="PSUM") as p3ps,
                ):
                    wout_sb = p2w.tile([128, HPC, D], bf16)
                    nc.sync.dma_start(
                        out=wout_sb,
                        in_=wout_d.rearrange("(c p) n -> p c n", p=128),
                    )
                    if not zero_bias:
                        bout_sb = p2w.tile([1, D], bf16)
                        nc.sync.dma_start(out=bout_sb, in_=bout_d[:, :])
                    maskT_r = [
                        maskT_d[h, :, :].rearrange("(kc p) q -> p kc q", p=128)
                        for h in range(HPC)
                    ]

                    # Software-pipelined sigmoid: group i+1's mask DMA and
                    # sigmoid are emitted before group i's exp so the ACT
                    # engine fills its wait-for-DVE gap with the next sigmoid
                    # (sig and exp live in different ACT LUT tables; this
                    # order also keeps table switches at 2 per group).
                    groups = [(qc, h) for qc in range(4) for h in range(HPC)]

                    def emit_mask_sig(qc, h):
                        qs = slice(qc * 512, (qc + 1) * 512)
                        mask_g = p2mask.tile([128, KC, 512], bf16, name="mask_g")
                        nc.sync.dma_start(out=mask_g, in_=maskT_r[h][:, :, qs])
                        sig_g = p2sig.tile([128, KC, 512], bf16, name="sig_g")
                        nc.scalar.activation(sig_g, mask_g, Act.Sigmoid)
                        return sig_g

                    def emit_outproj_chunk(qc, t2):
                        # one 128-query chunk of stripe qc's out-projection
                        t0 = qc * 512 + t2 * 128
                        for cc in range(4):  # 512-out-col chunks
                            cs = slice(cc * 512, (cc + 1) * 512)
                            ps = p3ps.tile([128, 512], f32, name="p3ps_t")
                            for hh in range(HPC):
                                nc.tensor.matmul(
                                    ps,
                                    lhsT=oT_sb[hh][:, t0 : t0 + 128],
                                    rhs=wout_sb[:, hh, cs],
                                    start=(hh == 0),
                                    stop=(hh == HPC - 1) and zero_bias,
                                )
                            if not zero_bias:
                                # bias (b_out/4 per core): ones x bias_row
                                nc.tensor.matmul(
                                    ps,
                                    lhsT=ones_bf[0:1, 0:128],
                                    rhs=bout_sb[0:1, cs],
                                    start=False,
                                    stop=True,
                                )
                            ost = p3s.tile([128, 512], f32, name="ost_t")
                            nc.vector.tensor_copy(ost, ps)
                            nc.sync.dma_start(
                                out=out_d[t0 : t0 + 128, cs], in_=ost
                            )

                    def emit_scores(qc, h):
                        # score matmuls only need qT/kT — emitted one group
                        # ahead so the in-order PE fills its exp-wait with
                        # them instead of stalling before den/av.
                        qs = slice(qc * 512, (qc + 1) * 512)
                        kT = qk_sb[4 + h]
                        qT = qk_sb[h]
                        tiles = []
                        for kc in range(16):
                            sps = spsp.tile([128, 512], f32, name="sps_t")
                            nc.tensor.matmul(
                                sps,
                                lhsT=kT[:, kc * 128 : (kc + 1) * 128],
                                rhs=qT[:, qs],
                                start=True,
                                stop=True,
                            )
                            tiles.append(sps)
                        return tiles

                    sig_next = emit_mask_sig(*groups[0])
                    sps_next = emit_scores(*groups[0])
                    deferred_norm = None  # (ops, bps, h, qs) from group i-1
                    for gi, (qc, h) in enumerate(groups):
                        qs = slice(qc * 512, (qc + 1) * 512)
                        sig_g = sig_next
                        sps_list = sps_next

                        # mskd in two half-tiles: exp of the first half can
                        # retire while the second half's muls still run, so
                        # the next group's muls only wait on a half-exp.
                        attn_g = p2attn.tile([128, KC, 512], bf16, name="attn_g")
                        half = KC // 2
                        for hf in range(2):
                            mskd_h = p2mskd.tile(
                                [128, half, 512], f32, name="mskd_h"
                            )
                            for k2 in range(half):
                                kc = hf * half + k2
                                nc.vector.tensor_mul(
                                    mskd_h[:, k2, :], sps_list[kc], sig_g[:, kc, :]
                                )
                            if hf == 0:
                                # next group's sigmoid fills ACT's mul-wait
                                if gi + 1 < len(groups):
                                    sig_next = emit_mask_sig(*groups[gi + 1])
                            nc.scalar.activation(
                                attn_g[:, hf * half : (hf + 1) * half, :],
                                mskd_h,
                                Act.Exp,
                            )

                        # previous group's normalization: its av matmuls are
                        # long done, so DVE doesn't stall here mid-pipeline.
                        if deferred_norm is not None:
                            d_ops, d_bps, d_h, d_qs = deferred_norm
                            d_rbs = p2r.tile([128, 512], f32, name="rbs_t")
                            nc.vector.tensor_copy(d_rbs, d_bps)
                            nc.vector.tensor_mul(
                                oT_sb[d_h][:, d_qs], d_ops, d_rbs
                            )

                        if gi + 1 < len(groups):
                            sps_next = emit_scores(*groups[gi + 1])

                        # denominator: sum over keys via ones-matmul
                        dps = dpsp.tile([1, 512], f32, name="dps_t")
                        for kc in range(16):
                            nc.tensor.matmul(
                                dps,
                                lhsT=ones_bf[:, 0:1],
                                rhs=attn_g[:, kc, :],
                                start=(kc == 0),
                                stop=(kc == 15),
                            )
                        # attn^T @ v -> out^T (accumulate over key chunks)
                        ops = opsp.tile([128, 512], f32, name="ops_t")
                        for kc in range(16):
                            nc.tensor.matmul(
                                ops,
                                lhsT=v_sb[:, kc, h * 128 : (h + 1) * 128],
                                rhs=attn_g[:, kc, :],
                                start=(kc == 0),
                                stop=(kc == 15),
                            )
                        # recip/broadcast last so neither engine stalls on
                        # them mid-stream (the norm that consumes them is
                        # deferred to the next iteration anyway)
                        rsb = p2r.tile([1, 512], f32, name="rsb_t")
                        nc.vector.reciprocal(rsb, dps)
                        bps = bpsp.tile([128, 512], f32, name="bps_t")
                        nc.tensor.matmul(
                            bps, lhsT=ones_f32, rhs=rsb, start=True, stop=True
                        )
                        deferred_norm = (ops, bps, h, qs)

                        # Spread the previous stripe's out-projection: one
                        # 128-query chunk after each group, so its matmuls
                        # don't block the next stripe's score matmuls on the
                        # in-order PE queue.
                        if qc > 0:
                            emit_outproj_chunk(qc - 1, h)

                    # drain the pipeline tail
                    d_ops, d_bps, d_h, d_qs = deferred_norm
                    d_rbs = p2r.tile([128, 512], f32, name="rbs_t")
                    nc.vector.tensor_copy(d_rbs, d_bps)
                    nc.vector.tensor_mul(oT_sb[d_h][:, d_qs], d_ops, d_rbs)
                    # last stripe's out-projection has no following groups
                    for t2 in range(4):
                        emit_outproj_chunk(3, t2)


def _prep_in_maps(x, W_qkv, b_qkv, W_out, b_out, causal_mask):
    from concurrent.futures import ThreadPoolExecutor

    import ml_dtypes

    bf = ml_dtypes.bfloat16

    def _xT(b):
        return np.ascontiguousarray(x[b].T).astype(bf)

    def _maskT(g):
        # cast to bf16 first (halves the bytes the transpose-copy moves)
        m = causal_mask[g * HPC : (g + 1) * HPC].astype(bf)
        return np.ascontiguousarray(m.transpose(0, 2, 1))

    with ThreadPoolExecutor(8) as ex:
        xT_f = [ex.submit(_xT, b) for b in range(B)]
        maskT_f = [ex.submit(_maskT, g) for g in range(4)]
        xT = [f.result() for f in xT_f]
        maskT = [f.result() for f in maskT_f]

    in_maps = []
    for c in range(NCORES):
        b = c // 4
        g = c % 4
        h0 = g * HPC  # first head of this core's group
        qcols = slice(h0 * HD, (h0 + HPC) * HD)
        kcols = slice(D + h0 * HD, D + (h0 + HPC) * HD)
        vcols = slice(2 * D + h0 * HD, 2 * D + (h0 + HPC) * HD)

        wqkv = np.concatenate(
            [
                W_qkv[:, qcols] * ALPHA,
                W_qkv[:, kcols],
                W_qkv[:, vcols],
            ],
            axis=1,
        )
        bqkv = np.concatenate(
            [b_qkv[qcols] * ALPHA, b_qkv[kcols], b_qkv[vcols]]
        ).reshape(1, -1)
        in_maps.append(
            {
                "xT": xT[b],
                "wqkv": wqkv.astype(bf),
                "bqkv": bqkv.astype(bf),
                "maskT": maskT[g],
                "wout": W_out[h0 * HD : (h0 + HPC) * HD, :].astype(bf),
                "bout": (b_out * 0.25).reshape(1, -1).astype(bf),
            }
        )
    return in_maps


def _zero_bias(b_qkv, b_out):
    return bool(not b_qkv.any() and not b_out.any())


def kernel(**inputs):
    x = np.asarray(inputs["x"], dtype=np.float32)
    W_qkv = np.asarray(inputs["W_qkv"], dtype=np.float32)
    b_qkv = np.asarray(inputs["b_qkv"], dtype=np.float32)
    W_out = np.asarray(inputs["W_out"], dtype=np.float32)
    b_out = np.asarray(inputs["b_out"], dtype=np.float32)
    causal_mask = np.asarray(inputs["causal_mask"], dtype=np.float32)

    from concourse.bass_utils import run_bass_kernel_spmd

    nc = _build_program(_zero_bias(b_qkv, b_out))
    in_maps = _prep_in_maps(x, W_qkv, b_qkv, W_out, b_out, causal_mask)
    res = run_bass_kernel_spmd(nc, in_maps, core_ids=list(range(NCORES)))

    out = np.zeros((B, N, D), dtype=np.float32)
    for c in range(NCORES):
        out[c // 4] += np.asarray(res.results[c]["out"], dtype=np.float32)
    return out
